# revision 62
# baseline (speedup 1.0000x reference)
"""Trainium2 Bass kernel for nn_CausalAttention (N=4096, 8 heads, DH=32).

Strategy: head-parallel across 8 NeuronCores (1 head per core).
Per core (v3):
  - bf16 inputs/projections (halved DMA + weight loads; PSUM accumulate f32).
  - Scores computed transposed: S^T[k, q] = K @ Q^T, 512-query blocks,
    3 k-tiles (128 keys) per PSUM group (GS=3, row-group packed K=32
    matmuls at partition offsets 0/32/64 pipeline back-to-back on the PE).
    Diagonal sub-tiles skip their fully-masked column prefix.
  - Max-free softmax: P^T = exp(S / sqrt(32)), strict-causal 0/1 mask
    post-exp (scores O(1): no overflow; -10000 masking underflows to 0).
  - Softmax denominator via ones column appended to V (lhsT [128, 33]).
  - V projection: 4 k-tiles batched into one PSUM bank, single DVE
    bias-add per 512-slice.
  - PV matmuls slice away the causally-dead query prefix on diagonal
    tiles; PV flushing crosses block boundaries (2-group lag) so the PE
    never bursts unpipelined at block ends.
  - Projection chunks (K/Q/V per slice) are spread between attention
    groups so their PSUM-slot serialization hides behind score/PV work.
  - DMA: first slices + weights interleaved at the head of both rings
    (sync HWDGE + gpsimd SWDGE round-robin per transfer); bulk slices
    follow in ring-FIFO order. The ACT queue carries only the exps.
  - o_ps tiles span 33 partitions, so adjacent blocks double-buffer inside
    ONE psum bank at partition offsets 0/64 (no block-boundary stall).
  - Normalization all on-chip, reading o_ps straight from PSUM: denom row
    -> (native add to SBUF) -> reciprocal_approx_fast [1,512] -> gpsimd
    partition_broadcast -> DVE multiply (PSUM read) -> bf16 out DMA,
    staged across following groups; the final block takes an immediate
    256-col-pipelined path. Output O^T [32, 4096] bf16; host reshapes.
"""

import math

import numpy as np
import ml_dtypes

import concourse.bass as bass
import concourse.mybir as mybir
from concourse import bacc
from concourse.tile import TileContext
from concourse.bass_utils import run_bass_kernel_spmd

# Problem constants (hardcoded per harness contract).
B, CQ, CK, CH, NH, H, W = 1, 256, 256, 256, 8, 64, 64
DH = CH // NH            # 32
N = H * W                # 4096
QB = 512                 # queries per block
NQB = N // QB            # 8
KT = 128                 # keys per k-tile
NKT = N // KT            # 32
GS = 3                   # k-tiles per S-group (3 PSUM banks per group)
NG = (NKT + GS - 1) // GS  # 11 column-groups in packed kT layout
SCALE = 1.0 / math.sqrt(DH)

F32 = mybir.dt.float32
BF16 = mybir.dt.bfloat16

_CACHED_NC = None


def _build():
    nc = bacc.Bacc("TRN2", target_bir_lowering=False, debug=False, num_devices=1)

    qin_d = nc.dram_tensor("qin", [CQ, N], BF16, kind="ExternalInput")
    kin_d = nc.dram_tensor("kin", [CK, N], BF16, kind="ExternalInput")
    wq_d = nc.dram_tensor("wqt", [CQ, 128], BF16, kind="ExternalInput")
    wk_d = nc.dram_tensor("wkt", [CK, 128], BF16, kind="ExternalInput")
    wv_d = nc.dram_tensor("wvt", [CK, DH], BF16, kind="ExternalInput")
    bq_d = nc.dram_tensor("bqr", [128, 1], F32, kind="ExternalInput")
    bk_d = nc.dram_tensor("bkr", [128, 1], F32, kind="ExternalInput")
    bv_d = nc.dram_tensor("bv4", [128, 128], F32, kind="ExternalInput")
    out_d = nc.dram_tensor("out", [DH, N], BF16, kind="ExternalOutput")

    # Strict-causal mask window: tm[kk, j] = 1.0 iff kk < j - 384; the
    # [*, 384:512] slice gives mask[kk, qq] = (kk < qq) for the 128-wide
    # diagonal window.
    tm_np = (np.arange(128)[:, None] < (np.arange(512)[None, :] - 384)).astype(
        ml_dtypes.bfloat16
    )
    tm_d = nc.inline_tensor(tm_np, name="tmask")

    with TileContext(nc) as tc:
        with (
            tc.tile_pool(name="constp", bufs=1) as constp,
            tc.tile_pool(name="bigp", bufs=1) as bigp,
            tc.tile_pool(name="workp", bufs=4) as workp,
            tc.tile_pool(name="spool", bufs=2, space="PSUM") as spool,
            tc.tile_pool(name="ppool", bufs=1, space="PSUM") as ppool,
            tc.tile_pool(name="opool", bufs=1, space="PSUM") as opool,
        ):
            kin_sb = bigp.tile([128, 2, N], BF16, name="kin_sb")
            qin_sb = bigp.tile([128, 2, N], BF16, name="qin_sb")
            kin_ap = kin_d.ap().rearrange("(c p) n -> p c n", p=128)
            qin_ap = qin_d.ap().rearrange("(c p) n -> p c n", p=128)

            def dma_slice(eng, which, s, ch=None):
                sl = slice(512 * s, 512 * (s + 1))
                c = slice(None) if ch is None else slice(ch, ch + 1)
                if which == "k":
                    eng.dma_start(kin_sb[:, c, sl], kin_ap[:, c, sl])
                else:
                    eng.dma_start(qin_sb[:, c, sl], qin_ap[:, c, sl])

            # ---- priority path split across both rings; bulk follows in
            # ring-FIFO order so it can't steal bus from the priority path ----
            # first slices + tiny weights interleave across both rings so the
            # K/Q projection chains unblock as early as possible
            dma_slice(nc.sync, "k", 0, ch=0)
            dma_slice(nc.gpsimd, "q", 0, ch=0)
            dma_slice(nc.sync, "k", 0, ch=1)
            dma_slice(nc.gpsimd, "q", 0, ch=1)
            wk_sb = constp.tile([128, 2, 128], BF16, name="wk_sb")
            nc.sync.dma_start(
                wk_sb[:], wk_d.ap().rearrange("(c p) m -> p c m", p=128)
            )
            bk_sb = constp.tile([128, 1], F32, name="bk_sb")
            nc.sync.dma_start(bk_sb[:], bk_d.ap())
            wq_sb = constp.tile([128, 2, 128], BF16, name="wq_sb")
            nc.gpsimd.dma_start(
                wq_sb[:], wq_d.ap().rearrange("(c p) m -> p c m", p=128)
            )
            bq_sb = constp.tile([128, 1], F32, name="bq_sb")
            nc.gpsimd.dma_start(bq_sb[:], bq_d.ap())
            wv_sb = constp.tile([128, 2, DH], BF16, name="wv_sb")
            nc.sync.dma_start(
                wv_sb[:], wv_d.ap().rearrange("(c p) m -> p c m", p=128)
            )
            dma_slice(nc.sync, "k", 1)
            dma_slice(nc.gpsimd, "q", 1)
            bv4_sb = constp.tile([128, 4, DH], F32, name="bv4_sb")
            nc.sync.dma_start(bv4_sb[:], bv_d.ap().rearrange("p (t d) -> p t d", t=4))
            tm_sb = constp.tile([128, 512], BF16, name="tm_sb")
            nc.sync.dma_start(tm_sb[:], tm_d.ap())
            # bulk slices 2-7
            for s in range(2, 8):
                dma_slice(nc.gpsimd if s % 2 else nc.sync, "k", s)
                dma_slice(nc.sync if s % 2 else nc.gpsimd, "q", s)

            # Warm the exp activation table before scores exist (bk arrives
            # on the first DMAs — don't gate the table load on late consts).
            warm = workp.tile([1, 1], F32, name="warm")
            nc.scalar.activation(
                warm[:], bk_sb[0:1, 0:1], mybir.ActivationFunctionType.Exp
            )

            # kT3[32u+d, 128g+kk] = k^T[d, 128*(3g+u)+kk]  (3-way row packing)
            kT3 = bigp.tile([32 * GS, NG * 128], BF16, name="kT3")
            # qT[32u+d, q] = q^T[d, q] for u=0..3 (4x replicated on partitions)
            qT = bigp.tile([128, N], BF16, name="qT")
            # v_all[kk, t, :DH] = v[128t+kk, :]; col DH is the ones column
            v_all = bigp.tile([128, NKT, 40], BF16, name="v_all")
            nc.vector.memset(v_all[:, :, DH : DH + 1], 1.0)

            # ---- projection chunks (emitted interleaved with attention) ----
            def proj_k(s):
                ksl = slice(512 * s, 512 * (s + 1))
                pj = ppool.tile([128, 512], F32, name="pj", tag="p")
                for ch in range(2):
                    nc.tensor.matmul(
                        pj[:],
                        wk_sb[:, ch, :],
                        kin_sb[:, ch, ksl],
                        start=(ch == 0),
                        stop=(ch == 1),
                    )
                for ci in range(4):
                    j = 4 * s + ci
                    u, g = j % GS, j // GS
                    nc.vector.tensor_scalar_add(
                        kT3[32 * u : 32 * u + 32, 128 * g : 128 * g + 128],
                        pj[32 * u : 32 * u + 32, 128 * ci : 128 * ci + 128],
                        bk_sb[32 * u : 32 * u + 32, :],
                    )

            def proj_q(s):
                ksl = slice(512 * s, 512 * (s + 1))
                # slice 0 uses the (still-free) opool bank so the K and Q
                # projection chains run in parallel at startup
                pool = opool if s == 0 else ppool
                tag = "o" if s == 0 else "p"
                pj = pool.tile([128, 512], F32, name="pj", tag=tag)
                for ch in range(2):
                    nc.tensor.matmul(
                        pj[:],
                        wq_sb[:, ch, :],
                        qin_sb[:, ch, ksl],
                        start=(ch == 0),
                        stop=(ch == 1),
                    )
                nc.vector.tensor_scalar_add(qT[:, ksl], pj[:], bq_sb[:])

            def proj_v(s):
                pj = ppool.tile([128, 4, DH], F32, name="pj", tag="p")
                for ti in range(4):
                    t = 4 * s + ti
                    nsl = slice(128 * t, 128 * (t + 1))
                    for ch in range(2):
                        nc.tensor.matmul(
                            pj[:, ti, :],
                            kin_sb[:, ch, nsl],
                            wv_sb[:, ch, :],
                            start=(ch == 0),
                            stop=(ch == 1),
                        )
                nc.vector.tensor_add(
                    v_all[:, 4 * s : 4 * s + 4, 0:DH], pj[:], bv4_sb[:]
                )

            # ---- tails (all on-chip; staged to hide latency) ----
            # tails read o_ps straight from PSUM (no copies); the +1e-30
            # divide-by-zero guard is only needed for block 0 (query 0 is
            # fully masked; every other block's denominator is positive)
            def tail_b(st):
                # custom-DVE (approx reciprocal) must read SBUF: stage the
                # denominator row through a native tensor_scalar_add
                cs_sb = workp.tile([1, 512], F32, name="cs_sb")
                nc.vector.tensor_scalar_add(
                    cs_sb[:], st["o_ps"][DH : DH + 1, :], 1e-30
                )
                csr = workp.tile([1, 512], F32, name="csr")
                nc.vector.reciprocal_approx_fast(csr[:], cs_sb[:])
                st.update(csr=csr)

            def tail_b2(st):
                rep = workp.tile([DH, 512], F32, name="rep")
                nc.gpsimd.partition_broadcast(rep[:], st["csr"][:])
                st.update(rep=rep)

            def tail_c(st):
                qb = st["qb"]
                out_sb = workp.tile([DH, 512], BF16, name="out_sb")
                nc.vector.tensor_mul(out_sb[:], st["o_ps"][0:DH, :], st["rep"][:])
                nc.sync.dma_start(
                    out_d.ap()[:, 512 * qb : 512 * (qb + 1)], out_sb[:]
                )

            # ---- global group stream with cross-block PV pends ----
            stage_q = []     # deferred tail stages, advanced one per group
            chunk_q = []     # pending projection chunks
            chunks_added = []  # slices whose proj chunks have been queued
            chunks_popped = [0]
            pends = []       # (qb, g, nsub, p_sb) awaiting PV
            ostate = {}      # qb -> {"o_ps": tile, "first": bool}

            def ngroups(qb):
                return (4 * (qb + 1) + GS - 1) // GS

            def flush_one():
                qb, g, nsub, p_sb = pends.pop(0)
                st = ostate.get(qb)
                if st is None:
                    # o_ps tiles only span 33 partitions, so adjacent blocks
                    # double-buffer within ONE psum bank at partition offsets
                    # 0/64 (subtile dep tracking keeps them independent)
                    off = 64 * (qb % 2)
                    st = ostate[qb] = {
                        "o_ps": o_base[off : off + DH + 1, :],
                        "first": True,
                    }
                o_ps = st["o_ps"]
                last_g = g == ngroups(qb) - 1
                for u in range(nsub):
                    j = GS * g + u
                    o = max(0, 128 * j - 512 * qb)
                    if st["first"]:
                        o = 0  # first matmul must initialize full PSUM
                    nc.tensor.matmul(
                        o_ps[:, o:512],
                        v_all[:, j, 0 : DH + 1],
                        p_sb[:, 512 * u + o : 512 * (u + 1)],
                        start=st["first"],
                        stop=(last_g and u == nsub - 1),
                        skip_group_check=True,
                    )
                    st["first"] = False
                if last_g:
                    ts = {"qb": qb, "o_ps": o_ps}
                    if qb == NQB - 1:
                        # final block: nothing left to overlap with — run the
                        # whole tail immediately, in 256-col halves pipelined
                        # across DVE / gpsimd / DMA to minimize drain latency
                        cs_sb = workp.tile([1, 512], F32, name="cs_sb")
                        csr = workp.tile([1, 512], F32, name="csr")
                        for c in range(2):
                            csl = slice(256 * c, 256 * (c + 1))
                            nc.vector.tensor_scalar_add(
                                cs_sb[:, csl], o_ps[DH : DH + 1, csl], 1e-30
                            )
                            nc.vector.reciprocal_approx_fast(
                                csr[:, csl], cs_sb[:, csl]
                            )
                        for c in range(2):
                            csl = slice(256 * c, 256 * (c + 1))
                            rep = workp.tile([DH, 256], F32, name="repc", bufs=2)
                            nc.gpsimd.partition_broadcast(rep[:], csr[:, csl])
                            out_sb = workp.tile(
                                [DH, 256], BF16, name="outc", bufs=2
                            )
                            nc.vector.tensor_mul(
                                out_sb[:], o_ps[0:DH, csl], rep[:]
                            )
                            nc.sync.dma_start(
                                out_d.ap()[:, 512 * qb + 256 * c :
                                           512 * qb + 256 * (c + 1)],
                                out_sb[:],
                            )
                    else:
                        stage_q.append(lambda ts=ts: tail_b(ts))
                        stage_q.append(lambda ts=ts: tail_b2(ts))
                        stage_q.append(lambda ts=ts: tail_c(ts))
                    del ostate[qb]

            def emit_group(qb, g):
                nkt_q = 4 * (qb + 1)
                nsub = min(GS, nkt_q - GS * g)
                s_ps = spool.tile([128, GS * 512], F32, name="s_ps", tag="s")
                for u in range(nsub):
                    j = GS * g + u
                    o = max(0, 128 * j - 512 * qb)
                    nc.tensor.matmul(
                        s_ps[:, 512 * u + o : 512 * (u + 1)],
                        kT3[32 * u : 32 * u + 32, 128 * g : 128 * g + 128],
                        qT[32 * u : 32 * u + 32, 512 * qb + o : 512 * (qb + 1)],
                        start=True,
                        stop=True,
                    )
                p_sb = workp.tile([128, GS * 512], BF16, name="p_sb", bufs=8)
                if qb == NQB - 1 and g == ngroups(qb) - 1:
                    # the very last group heads the exposed drain chain:
                    # per-subtile exp + mask so the first PV starts sooner
                    for u in range(nsub):
                        j = GS * g + u
                        o = 128 * j - 512 * qb
                        op = max(0, o)
                        nc.scalar.activation(
                            p_sb[:, 512 * u + op : 512 * (u + 1)],
                            s_ps[:, 512 * u + op : 512 * (u + 1)],
                            mybir.ActivationFunctionType.Exp,
                            scale=SCALE,
                        )
                        if o >= 0:
                            nc.vector.tensor_mul(
                                p_sb[:, 512 * u + o : 512 * u + o + 128],
                                p_sb[:, 512 * u + o : 512 * u + o + 128],
                                tm_sb[:, 384:512],
                            )
                    pends.append((qb, g, nsub, p_sb))
                    return
                # leading causally-dead columns of sub-tile 0 are never read
                # by the PV matmuls (they slice [o:512]), so skip their exp
                o0 = max(0, 128 * GS * g - 512 * qb)
                nc.scalar.activation(
                    p_sb[:, o0 : 512 * nsub],
                    s_ps[:, o0 : 512 * nsub],
                    mybir.ActivationFunctionType.Exp,
                    scale=SCALE,
                )
                for u in range(nsub):
                    j = GS * g + u
                    o = 128 * j - 512 * qb
                    if o >= 0:  # strict-causal mask on the diagonal window
                        nc.vector.tensor_mul(
                            p_sb[:, 512 * u + o : 512 * u + o + 128],
                            p_sb[:, 512 * u + o : 512 * u + o + 128],
                            tm_sb[:, 384:512],
                        )
                pends.append((qb, g, nsub, p_sb))

            # prefill: slice-0 projections, then stream blocks with the next
            # slice's chunks spread across group boundaries
            proj_k(0)
            proj_q(0)
            proj_v(0)
            # single psum bank shared by all blocks' o_ps (allocated after
            # proj_q(0)'s opool use so the slot rotation stays acyclic)
            o_base = opool.tile([128, 512], F32, name="o_base", tag="o")
            for qb in range(NQB):
                # slices 1-3 arrive just in time (chunked per block); by
                # block 2 all inputs have landed, so front-load the rest —
                # projections then finish by ~block 5, and late blocks never
                # wait on kT3/qT/v_all writes
                hi = NQB if qb == 3 else min(qb + 2, NQB)
                while len(chunks_added) < hi:
                    s = len(chunks_added)
                    chunks_added.append(s)
                    if s == 0:
                        continue
                    chunk_q.extend(
                        [
                            lambda s=s: proj_k(s),
                            lambda s=s: proj_q(s),
                            lambda s=s: proj_v(s),
                        ]
                    )
                thr = 1 if qb == NQB - 1 else 2  # drain tighter at the end
                for g in range(ngroups(qb)):
                    emit_group(qb, g)
                    # PV flush first: proj matmuls can stall on the ppool
                    # chain and must not head-block ready PV work in the
                    # in-order PE queue
                    if len(pends) > thr:
                        flush_one()
                    if chunk_q:  # proj adds get DVE priority over tail ops
                        chunks_popped[0] += 1
                        chunk_q.pop(0)()
                    if g == 0 and chunk_q:  # K+Q chunks both in group 0
                        chunks_popped[0] += 1
                        chunk_q.pop(0)()
                    if stage_q:
                        stage_q.pop(0)()
                # small early blocks: make sure the next block's slice is
                # fully projected (don't burst-drain the rest)
                while chunk_q and chunks_popped[0] < 3 * (qb + 1):
                    chunks_popped[0] += 1
                    chunk_q.pop(0)()
            while pends:
                flush_one()
            while stage_q:
                stage_q.pop(0)()

    nc.finalize()
    return nc


def _get_nc():
    global _CACHED_NC
    if _CACHED_NC is None:
        _CACHED_NC = _build()
    return _CACHED_NC


def _prep_in_maps(inputs):
    f = lambda a: np.ascontiguousarray(np.asarray(a, dtype=np.float32))
    bf = lambda a: np.ascontiguousarray(a.astype(ml_dtypes.bfloat16))
    query = bf(f(inputs["query"]).reshape(CQ, N))
    key_feat = bf(f(inputs["key_feat"]).reshape(CK, N))

    def wnorm(v, g):
        v = f(v)
        g = f(g)
        return g[:, None] * v / np.linalg.norm(v, axis=1, keepdims=True)

    wq = wnorm(inputs["vq"], inputs["gq"])
    wk = wnorm(inputs["vk"], inputs["gk"])
    wv = wnorm(inputs["vv"], inputs["gv"])
    bq, bk, bv = f(inputs["bq"]), f(inputs["bk"]), f(inputs["bv"])

    in_maps = []
    for c in range(NH):
        rows = slice(DH * c, DH * (c + 1))
        in_maps.append(
            {
                "qin": query,
                "kin": key_feat,
                "wqt": bf(np.tile(wq[rows].T, (1, 4))),
                "wkt": bf(np.tile(wk[rows].T, (1, 4))),
                "wvt": bf(wv[rows].T),
                "bqr": np.ascontiguousarray(np.tile(bq[rows], 4)[:, None]),
                "bkr": np.ascontiguousarray(np.tile(bk[rows], 4)[:, None]),
                "bv4": np.ascontiguousarray(
                    np.tile(bv[rows], (128, 4)).astype(np.float32)
                ),
            }
        )
    return in_maps


def _run(inputs, trace=False, **kwargs):
    nc = _get_nc()
    in_maps = _prep_in_maps(inputs)
    res = None
    for attempt in range(3):
        try:
            res = run_bass_kernel_spmd(
                nc, in_maps, core_ids=list(range(NH)), trace=trace, **kwargs
            )
            break
        except Exception:
            if attempt == 2:
                raise

    out = np.empty((B, CH, H, W), dtype=np.float32)
    for c in range(NH):
        oc = np.asarray(res.results[c]["out"], dtype=np.float32)  # [DH, N]
        out[0, DH * c : DH * (c + 1)] = oc.reshape(DH, H, W)
    return out, res


def kernel(**inputs) -> np.ndarray:
    out, _ = _run(inputs, trace=False)
    return out
